# revision 20
# baseline (speedup 1.0000x reference)
"""Trainium2 Bass kernel for the pairwise-MLP GNN message-passing problem.

dro[b,n,m] = W3 . relu(W2^T relu(PhiA[b,n] @ W1a + PhiB[b,m] @ W1b + b1) + b2) + b3

Shapes (hardcoded): B=2, N=1024, M=256, D=576 (padded to 640), H1=512, H2=256.
Sharding: 8 cores over (B, N): core c handles b = c//4, n in [256*(c%4), 256*(c%4)+256).
Weights replicated. Each core computes its (256, 256) tile of dro independently.

Two kernel layouts, chosen at runtime:

v4 (used when b2 == 0, which holds for this problem's inputs):
- z2 computed TRANSPOSED: for each (m, n-block-of-128), stationary = h1 block
  [128 h1-slice, 128 n], moving = W2s [128, 256 h2] where W2s has |w3| folded
  into its columns (relu(c*x) = c*relu(x)) and columns sorted by sign(w3).
- layer 3 then is a signed reduce along the FREE dim of z2T [128 n, 256 h2]:
  DVE scalar_tensor_tensor (relu * sign, accum_out) for ~30% of blocks,
  ACT activation(Relu, accum_out) x2 (positive/negative column ranges) for the
  rest.  The PE never runs layer 3: PE work = layer2 (218us) + layer1 (4us).
- dro columns assemble naturally into [128 n, 256 m] SBUF tiles -> 2 DMAs.

e1a (fallback for arbitrary b2):
- h1 = relu(bplus + a_col) on DVE in bf16, layer 2 straight (h2 on partitions,
  pairs on free), relu2 = relu(z2+b2) via ACT bias, layer 3 on the PE.
"""

import os
import numpy as np
import ml_dtypes

B, N, M = 2, 1024, 256
D, D_PAD = 576, 640
H1, H2 = 512, 256
N_CORES = 8
N_LOC = N // 4          # 256 rows of dro per core
KT1 = D_PAD // 128      # 5 contraction tiles for layer 1
KT2 = H1 // 128         # 4 contraction tiles for layer 2
H1T = H1 // 128         # 4 partition tiles of h1
H2T = H2 // 128         # 2 partition tiles of h2
NB = N_LOC // 128       # 2 n-blocks of 128 (v4)
CHUNK_N = 2             # robot points per inner chunk (e1a)
PAIRS = CHUNK_N * M     # 512
N_CHUNKS = N_LOC // CHUNK_N   # 128

_RUNNER = None
_PATH = None            # 'v4' or 'e1a', set by build_per_core
_POS_CNT = None         # number of w3>=0 columns after the sign sort (v4)


def _round_fp32r(a):
    b = np.ascontiguousarray(a, dtype=np.float32).view(np.uint32)
    return ((b + np.uint32(0x800)) & np.uint32(0xFFFFF000)).view(np.float32)


def _to_bf16(a):
    return np.asarray(a, dtype=np.float32).astype(ml_dtypes.bfloat16)


def _split_multiwaits(bir_json):
    """This container's walrus accepts only one sync-wait command per
    instruction; hoist all but the last wait onto preceding same-engine
    EventSemaphore instructions (semantically identical: consecutive waits)."""
    import orjson

    d = orjson.loads(bir_json)
    for fn in d.get("functions", []):
        for blk in fn.get("blocks", []):
            insts = blk.get("instructions") or []
            out = []
            for inst in insts:
                si = inst.get("sync_info")
                waits = (si or {}).get("on_wait") or []
                if len(waits) > 1:
                    for j, w in enumerate(waits[:-1]):
                        out.append({
                            "debug": inst.get("debug", 0),
                            "engine": inst["engine"],
                            "ins": [],
                            "name": f"{inst['name']}-mw{j}",
                            "opcode": "EventSemaphore",
                            "outs": [],
                            "sync_info": {"on_update": [], "on_wait": [w]},
                        })
                    si["on_wait"] = [waits[-1]]
                out.append(inst)
            blk["instructions"] = out
    return orjson.dumps(d)


def _install_birfix():
    import concourse.bass2jax as b2j

    if getattr(b2j, "_multiwait_patched", False):
        return
    orig = b2j.compile_bir_kernel

    def patched(bir_json, tmpdir, neff_name="file.neff"):
        return orig(_split_multiwaits(bir_json), tmpdir, neff_name=neff_name)

    b2j.compile_bir_kernel = patched
    b2j._multiwait_patched = True


def _build_nc(repeat=1):
    if _PATH == "v4":
        return _build_nc_v4(repeat=repeat, pos=_POS_CNT)
    return _build_nc_e1a(repeat=repeat)


def _build_nc_v4(repeat=1, pos=128):
    import concourse.bass as bass
    import concourse.tile as tile
    import concourse.mybir as mybir

    f32 = mybir.dt.float32
    f32r = mybir.dt.float32r
    bf16 = mybir.dt.bfloat16
    add_op = mybir.AluOpType.add
    max_op = mybir.AluOpType.max
    mult_op = mybir.AluOpType.mult
    sub_op = mybir.AluOpType.subtract
    relu_fn = mybir.ActivationFunctionType.Relu

    nc = bass.Bass("TRN2", target_bir_lowering=False, debug=False,
                   num_devices=N_CORES)

    pa_ext = nc.dram_tensor("pa_t", [D_PAD, N_LOC], f32r, kind="ExternalInput")
    pb_ext = nc.dram_tensor("pb_t", [D_PAD, M], f32r, kind="ExternalInput")
    w1a_ext = nc.dram_tensor("w1a", [D_PAD, H1], f32r, kind="ExternalInput")
    w1b_ext = nc.dram_tensor("w1b", [D_PAD, H1], f32r, kind="ExternalInput")
    w2s_ext = nc.dram_tensor("w2s", [H1, H2], bf16, kind="ExternalInput")
    sgn_ext = nc.dram_tensor("sgn", [128, H2], f32, kind="ExternalInput")
    b1_ext = nc.dram_tensor("b1c", [H1, 1], f32, kind="ExternalInput")
    dro_ext = nc.dram_tensor("dro", [NB, 128, M], f32, kind="ExternalOutput")

    with tile.TileContext(nc) as tc:
        with tc.tile_pool(name="consts", bufs=1) as consts, \
             tc.tile_pool(name="proj", bufs=1) as proj:

            pa_sb, pb_sb, w1a_sb, w1b_sb = [], [], [], []
            for kt in range(KT1):
                t = consts.tile([128, N_LOC], f32r, tag=f"pa{kt}")
                nc.sync.dma_start(out=t, in_=pa_ext[kt * 128:(kt + 1) * 128, :])
                pa_sb.append(t)
                t = consts.tile([128, M], f32r, tag=f"pb{kt}")
                nc.sync.dma_start(out=t, in_=pb_ext[kt * 128:(kt + 1) * 128, :])
                pb_sb.append(t)
                t = consts.tile([128, H1], f32r, tag=f"w1a{kt}")
                nc.sync.dma_start(out=t, in_=w1a_ext[kt * 128:(kt + 1) * 128, :])
                w1a_sb.append(t)
                t = consts.tile([128, H1], f32r, tag=f"w1b{kt}")
                nc.sync.dma_start(out=t, in_=w1b_ext[kt * 128:(kt + 1) * 128, :])
                w1b_sb.append(t)
            w2s_sb = []
            for kt in range(KT2):
                t = consts.tile([128, H2], bf16, tag=f"w2s{kt}")
                nc.sync.dma_start(out=t, in_=w2s_ext[kt * 128:(kt + 1) * 128, :])
                w2s_sb.append(t)
            sgn_sb = consts.tile([128, H2], f32, tag="sgn")
            nc.sync.dma_start(out=sgn_sb, in_=sgn_ext[:, :])
            b1_sb = []
            for ht in range(H1T):
                t = consts.tile([128, 1], f32, tag=f"b1{ht}")
                nc.sync.dma_start(out=t, in_=b1_ext[ht * 128:(ht + 1) * 128, :])
                b1_sb.append(t)

            # persistent dro accumulators: dpos gets the positive-range sums
            # (or the full signed sum for DVE-handled blocks), dneg the
            # negative-range sums (stays 0 for DVE-handled columns)
            dpos_sb, dneg_sb, dout_sb = [], [], []
            for nb in range(NB):
                t = proj.tile([128, M], f32, tag=f"dp{nb}", name=f"dp{nb}")
                dpos_sb.append(t)
                t = proj.tile([128, M], f32, tag=f"dn{nb}", name=f"dn{nb}")
                nc.vector.memset(t, 0.0)
                dneg_sb.append(t)
                t = proj.tile([128, M], f32, tag=f"do{nb}", name=f"do{nb}")
                dout_sb.append(t)

            # ---- stage A: a_proj (bf16, streamed by DVE) and
            # bplus = b_proj + b1 (f32: tensor_scalar scalar operand) ----
            a_proj = []
            bplus = []
            with tc.tile_pool(name="apsum", bufs=2, space="PSUM") as apsum:
                for ht in range(H1T):
                    ps = apsum.tile([128, N_LOC], f32, tag="ps")
                    for kt in range(KT1):
                        nc.tensor.matmul(
                            ps, w1a_sb[kt][:, ht * 128:(ht + 1) * 128],
                            pa_sb[kt], start=(kt == 0), stop=(kt == KT1 - 1))
                    t = proj.tile([128, N_LOC], bf16, tag=f"ap{ht}")
                    nc.scalar.copy(t, ps)
                    a_proj.append(t)
                for ht in range(H1T):
                    ps = apsum.tile([128, M], f32, tag="ps")
                    for kt in range(KT1):
                        nc.tensor.matmul(
                            ps, w1b_sb[kt][:, ht * 128:(ht + 1) * 128],
                            pb_sb[kt], start=(kt == 0), stop=(kt == KT1 - 1))
                    t = proj.tile([128, M], f32, tag=f"bp{ht}")
                    nc.scalar.activation(
                        t, ps, mybir.ActivationFunctionType.Identity,
                        bias=b1_sb[ht])
                    bplus.append(t)

            # ---- stage B: loop over m; per m one h1 tile [h1, 256 n] and
            # two z2T psum tiles [128 n, 256 h2] ----
            with tc.tile_pool(name="hpool", bufs=3) as hpool, \
                 tc.tile_pool(name="scr", bufs=4) as scr, \
                 tc.tile_pool(name="zpsum", bufs=3, space="PSUM") as zpsum:
                for m_rep in range(repeat * M):
                    m = m_rep % M
                    # h1_m[k, n] = relu(a_proj[k, n] + bplus[k, m])  (bf16)
                    h1 = []
                    for kt in range(KT2):
                        t = hpool.tile([128, N_LOC], bf16, tag=f"h1_{kt}")
                        nc.vector.tensor_scalar(
                            out=t, in0=a_proj[kt],
                            scalar1=bplus[kt][:, m:m + 1],
                            scalar2=0.0, op0=add_op, op1=max_op)
                        h1.append(t)
                    for nb in range(NB):
                        zps = zpsum.tile([128, H2], f32, tag=f"z{nb}")
                        for kt in range(KT2):
                            nc.tensor.matmul(
                                zps, h1[kt][:, nb * 128:(nb + 1) * 128],
                                w2s_sb[kt], start=(kt == 0),
                                stop=(kt == KT2 - 1))
                        # signed reduce of relu(z2T) over h2 -> dro column.
                        # Two flavors, mixed 1:1 so ACT and DVE both land at
                        # ~235us next to the PE's 224us:
                        #  A: ACT relu+accum over pos/neg column ranges
                        #  C: DVE signed-reduce straight from PSUM (1x)
                        kind = (m * NB + nb) % 16
                        if kind < 8:
                            s = scr.tile([128, H2], bf16, tag="s")
                            if pos > 0:
                                nc.scalar.activation(
                                    s[:, 0:pos], zps[:, 0:pos], relu_fn,
                                    accum_out=dpos_sb[nb][:, m:m + 1])
                            else:
                                nc.vector.memset(dpos_sb[nb][:, m:m + 1], 0.0)
                            if pos < H2:
                                nc.scalar.activation(
                                    s[:, pos:H2], zps[:, pos:H2], relu_fn,
                                    accum_out=dneg_sb[nb][:, m:m + 1])
                        else:
                            s = scr.tile([128, H2], bf16, tag="s")
                            nc.vector.scalar_tensor_tensor(
                                out=s, in0=zps, scalar=0.0, in1=sgn_sb,
                                op0=max_op, op1=mult_op,
                                accum_out=dpos_sb[nb][:, m:m + 1])
                    if m == M - 1:
                        for nb in range(NB):
                            nc.vector.tensor_tensor(
                                out=dout_sb[nb], in0=dpos_sb[nb],
                                in1=dneg_sb[nb], op=sub_op)
                            nc.sync.dma_start(out=dro_ext[nb],
                                              in_=dout_sb[nb])
    return nc


def _build_nc_e1a(repeat=1):
    import concourse.bass as bass
    import concourse.tile as tile
    import concourse.mybir as mybir

    f32 = mybir.dt.float32
    f32r = mybir.dt.float32r
    bf16 = mybir.dt.bfloat16
    add_op = mybir.AluOpType.add
    max_op = mybir.AluOpType.max

    nc = bass.Bass("TRN2", target_bir_lowering=False, debug=False,
                   num_devices=N_CORES)

    pa_ext = nc.dram_tensor("pa_t", [D_PAD, N_LOC], f32r, kind="ExternalInput")
    pb_ext = nc.dram_tensor("pb_t", [D_PAD, M], f32r, kind="ExternalInput")
    w1a_ext = nc.dram_tensor("w1a", [D_PAD, H1], f32r, kind="ExternalInput")
    w1b_ext = nc.dram_tensor("w1b", [D_PAD, H1], f32r, kind="ExternalInput")
    w2_ext = nc.dram_tensor("w2", [H1, H2], bf16, kind="ExternalInput")
    w3_ext = nc.dram_tensor("w3", [H2, 1], bf16, kind="ExternalInput")
    b1_ext = nc.dram_tensor("b1c", [H1, 1], f32, kind="ExternalInput")
    b2_ext = nc.dram_tensor("b2c", [H2, 1], f32, kind="ExternalInput")
    dro_ext = nc.dram_tensor("dro", [1, N_LOC * M], f32,
                             kind="ExternalOutput")

    with tile.TileContext(nc) as tc:
        with tc.tile_pool(name="consts", bufs=1) as consts, \
             tc.tile_pool(name="proj", bufs=1) as proj:

            # ---- load constants ----
            pa_sb = []
            pb_sb = []
            w1a_sb = []
            w1b_sb = []
            for kt in range(KT1):
                t = consts.tile([128, N_LOC], f32r, tag=f"pa{kt}")
                nc.sync.dma_start(out=t, in_=pa_ext[kt * 128:(kt + 1) * 128, :])
                pa_sb.append(t)
                t = consts.tile([128, M], f32r, tag=f"pb{kt}")
                nc.sync.dma_start(out=t, in_=pb_ext[kt * 128:(kt + 1) * 128, :])
                pb_sb.append(t)
                t = consts.tile([128, H1], f32r, tag=f"w1a{kt}")
                nc.sync.dma_start(out=t, in_=w1a_ext[kt * 128:(kt + 1) * 128, :])
                w1a_sb.append(t)
                t = consts.tile([128, H1], f32r, tag=f"w1b{kt}")
                nc.sync.dma_start(out=t, in_=w1b_ext[kt * 128:(kt + 1) * 128, :])
                w1b_sb.append(t)
            w2_sb = []
            for kt in range(KT2):
                t = consts.tile([128, H2], bf16, tag=f"w2{kt}")
                nc.sync.dma_start(out=t, in_=w2_ext[kt * 128:(kt + 1) * 128, :])
                w2_sb.append(t)
            b1_sb = []
            for ht in range(H1T):
                t = consts.tile([128, 1], f32, tag=f"b1{ht}")
                nc.sync.dma_start(out=t, in_=b1_ext[ht * 128:(ht + 1) * 128, :])
                b1_sb.append(t)
            b2_sb = []
            w3_sb = []
            for ht in range(H2T):
                t = consts.tile([128, 1], f32, tag=f"b2{ht}")
                nc.sync.dma_start(out=t, in_=b2_ext[ht * 128:(ht + 1) * 128, :])
                b2_sb.append(t)
                t = consts.tile([128, 1], bf16, tag=f"w3{ht}")
                nc.sync.dma_start(out=t, in_=w3_ext[ht * 128:(ht + 1) * 128, :])
                w3_sb.append(t)

            # ---- stage A ----
            a_proj = []
            bplus = []
            with tc.tile_pool(name="apsum", bufs=2, space="PSUM") as apsum:
                for ht in range(H1T):
                    ps = apsum.tile([128, N_LOC], f32, tag="ps")
                    for kt in range(KT1):
                        nc.tensor.matmul(
                            ps, w1a_sb[kt][:, ht * 128:(ht + 1) * 128],
                            pa_sb[kt], start=(kt == 0), stop=(kt == KT1 - 1))
                    t = proj.tile([128, N_LOC], f32, tag=f"ap{ht}")
                    nc.scalar.copy(t, ps)
                    a_proj.append(t)
                for ht in range(H1T):
                    ps = apsum.tile([128, M], f32, tag="ps")
                    for kt in range(KT1):
                        nc.tensor.matmul(
                            ps, w1b_sb[kt][:, ht * 128:(ht + 1) * 128],
                            pb_sb[kt], start=(kt == 0), stop=(kt == KT1 - 1))
                    t = proj.tile([128, M], bf16, tag=f"bp{ht}")
                    nc.scalar.activation(
                        t, ps, mybir.ActivationFunctionType.Identity,
                        bias=b1_sb[ht])
                    bplus.append(t)

            # ---- stage B ----
            with tc.tile_pool(name="hpool", bufs=8) as hpool, \
                 tc.tile_pool(name="rpool", bufs=4) as rpool, \
                 tc.tile_pool(name="spool", bufs=2) as spool, \
                 tc.tile_pool(name="zpsum", bufs=2, space="PSUM") as zpsum, \
                 tc.tile_pool(name="dpsum", bufs=2, space="PSUM") as dpsum:
                stg = None
                for c_rep in range(repeat * N_CHUNKS):
                    c = c_rep % N_CHUNKS
                    h1 = []
                    for kt in range(KT2):
                        t = hpool.tile([128, PAIRS], bf16, tag=f"h1_{kt}")
                        for half in range(CHUNK_N):
                            n = CHUNK_N * c + half
                            nc.vector.tensor_scalar(
                                out=t[:, half * M:(half + 1) * M],
                                in0=bplus[kt],
                                scalar1=a_proj[kt][:, n:n + 1],
                                scalar2=0.0,
                                op0=add_op, op1=max_op)
                        h1.append(t)
                    relu2 = []
                    for ht in range(H2T):
                        zps = zpsum.tile([128, PAIRS], f32, tag=f"z{ht}")
                        for kt in range(KT2):
                            nc.tensor.matmul(
                                zps, w2_sb[kt][:, ht * 128:(ht + 1) * 128],
                                h1[kt], start=(kt == 0), stop=(kt == KT2 - 1))
                        r = rpool.tile([128, PAIRS], bf16, tag=f"r{ht}")
                        nc.scalar.activation(
                            r, zps, mybir.ActivationFunctionType.Relu,
                            bias=b2_sb[ht])
                        relu2.append(r)
                    dps = dpsum.tile([1, PAIRS], f32, tag="d")
                    for ht in range(H2T):
                        nc.tensor.matmul(dps, w3_sb[ht], relu2[ht],
                                         start=(ht == 0), stop=(ht == H2T - 1))
                    s = c % 8
                    if s == 0:
                        stg = spool.tile([1, 8 * PAIRS], f32, tag="stg")
                    if c % 2 == 0:
                        nc.scalar.copy(stg[:, s * PAIRS:(s + 1) * PAIRS], dps)
                    else:
                        nc.vector.tensor_copy(
                            out=stg[:, s * PAIRS:(s + 1) * PAIRS], in_=dps)
                    if s == 7:
                        g = c // 8
                        sz = 8 * PAIRS
                        nc.sync.dma_start(
                            out=dro_ext[:, g * sz:(g + 1) * sz], in_=stg)
    return nc


class _Runner:
    def __init__(self, repeat=1):
        _install_birfix()
        import jax
        import numpy as _np
        from jax.sharding import Mesh, PartitionSpec
        from jax.experimental.shard_map import shard_map
        import concourse.bass2jax as b2j
        import concourse.mybir as mybir

        nc = _build_nc(repeat=repeat)
        self.nc = nc
        b2j.install_neuronx_cc_hook()

        partition_name = (nc.partition_id_tensor.name
                          if nc.partition_id_tensor else None)
        in_names, out_names, out_avals, zero_outs = [], [], [], []
        for alloc in nc.m.functions[0].allocations:
            if not isinstance(alloc, mybir.MemoryLocationSet):
                continue
            name = alloc.memorylocations[0].name
            if alloc.kind == "ExternalInput":
                if name != partition_name:
                    in_names.append(name)
            elif alloc.kind == "ExternalOutput":
                shape = tuple(alloc.tensor_shape)
                dtype = mybir.dt.np(alloc.dtype)
                out_names.append(name)
                out_avals.append(jax.core.ShapedArray(shape, dtype))
                zero_outs.append(_np.zeros(shape, dtype))
        n_params = len(in_names)
        self.in_names = list(in_names)
        self.out_names = out_names
        self.zero_outs = zero_outs
        bind_names = list(in_names) + list(out_names)
        if partition_name is not None:
            bind_names.append(partition_name)

        def _body(*args):
            operands = list(args)
            if partition_name is not None:
                operands.append(b2j.partition_id_tensor())
            outs = b2j._bass_exec_p.bind(
                *operands,
                out_avals=tuple(out_avals),
                in_names=tuple(bind_names),
                out_names=tuple(out_names),
                lowering_input_output_aliases=(),
                sim_require_finite=True,
                sim_require_nnan=True,
                nc=nc,
            )
            return tuple(outs)

        devices = jax.devices()[:N_CORES]
        assert len(devices) == N_CORES, f"need {N_CORES} cores, have {devices}"
        mesh = Mesh(_np.asarray(devices), ("core",))
        n_outs = len(out_names)
        self.fn = jax.jit(
            shard_map(_body, mesh=mesh,
                      in_specs=(PartitionSpec("core"),) * (n_params + n_outs),
                      out_specs=(PartitionSpec("core"),) * n_outs,
                      check_rep=False),
            keep_unused=True,
        )
        self.jax = jax

    def run(self, per_core_maps):
        np_ = np
        concat_in = [
            np_.concatenate([m[name] for m in per_core_maps], axis=0)
            for name in self.in_names
        ]
        concat_zero = [
            np_.zeros((N_CORES * z.shape[0], *z.shape[1:]), z.dtype)
            for z in self.zero_outs
        ]
        out = self.fn(*concat_in, *concat_zero)
        out = [np_.asarray(o) for o in out]
        return out

    def time_ns(self, per_core_maps, iters=10):
        import time
        jax = self.jax
        concat_in = [
            np.concatenate([m[name] for m in per_core_maps], axis=0)
            for name in self.in_names
        ]
        concat_zero = [
            np.zeros((N_CORES * z.shape[0], *z.shape[1:]), z.dtype)
            for z in self.zero_outs
        ]
        dev_in = [jax.device_put(a) for a in concat_in]
        dev_zero = [jax.device_put(a) for a in concat_zero]
        r = self.fn(*dev_in, *dev_zero)
        jax.block_until_ready(r)
        best = float("inf")
        for _ in range(iters):
            t0 = time.perf_counter_ns()
            r = self.fn(*dev_in, *dev_zero)
            jax.block_until_ready(r)
            dt = time.perf_counter_ns() - t0
            best = min(best, dt)
        return best


def _get_runner():
    global _RUNNER
    if _RUNNER is None:
        _RUNNER = _Runner()
    return _RUNNER


def build_per_core(inputs):
    """Shard + lay out the full inputs into per-core input maps.  Also decides
    which kernel layout to use (v4 requires b2 == 0) and sets _PATH/_POS_CNT."""
    global _PATH, _POS_CNT
    Phi_A = np.asarray(inputs["Phi_A"], dtype=np.float32)
    Phi_B = np.asarray(inputs["Phi_B"], dtype=np.float32)
    W1a = np.asarray(inputs["W1a"], dtype=np.float32)
    W1b = np.asarray(inputs["W1b"], dtype=np.float32)
    W2 = np.asarray(inputs["W2"], dtype=np.float32)
    W3 = np.asarray(inputs["W3"], dtype=np.float32)
    b1 = np.asarray(inputs["b1"], dtype=np.float32)
    b2 = np.asarray(inputs["b2"], dtype=np.float32)

    path = "e1a"  # v4 (transposed) loses on HW: per-matmul stationary reloads
    if _PATH is None:
        _PATH = path
    else:
        assert _PATH == path, "kernel layout fixed after first call"

    w1a_p = np.zeros((D_PAD, H1), np.float32)
    w1a_p[:D] = W1a
    w1b_p = np.zeros((D_PAD, H1), np.float32)
    w1b_p[:D] = W1b
    w1a_p = _round_fp32r(w1a_p)
    w1b_p = _round_fp32r(w1b_p)
    b1c = b1.reshape(H1, 1)

    com = {
        "w1a": w1a_p,
        "w1b": w1b_p,
        "b1c": b1c,
    }
    w3f = W3.reshape(H2)
    if path == "v4":
        # fold |w3| into W2 columns (relu(c*x) = c*relu(x), c >= 0), sort
        # columns so w3 >= 0 comes first; sign row handles the subtraction
        perm = np.argsort(w3f < 0, kind="stable")
        pos = int((w3f >= 0).sum())
        w2s = (W2 * np.abs(w3f)[None, :])[:, perm]
        sgn = np.sign(w3f)[perm]
        com["w2s"] = _to_bf16(w2s)
        com["sgn"] = np.ascontiguousarray(
            np.broadcast_to(sgn, (128, H2)), dtype=np.float32)
        _POS_CNT = pos
    else:
        com["w2"] = _to_bf16(W2)
        com["w3"] = _to_bf16(W3.reshape(H2, 1))
        com["b2c"] = b2.reshape(H2, 1)

    per_core = []
    for c in range(N_CORES):
        b = c // 4
        n0 = (c % 4) * N_LOC
        pa = np.zeros((D_PAD, N_LOC), np.float32)
        pa[:D] = Phi_A[b, n0:n0 + N_LOC, :].T
        pb = np.zeros((D_PAD, M), np.float32)
        pb[:D] = Phi_B[b].T
        per_core.append(dict(com, pa_t=_round_fp32r(pa), pb_t=_round_fp32r(pb)))
    return per_core


def kernel(Phi_A, Phi_B, W1a, W1b, b1, W2, b2, W3, b3):
    b3 = np.asarray(b3, dtype=np.float32)
    per_core = build_per_core({
        "Phi_A": Phi_A, "Phi_B": Phi_B, "W1a": W1a, "W1b": W1b,
        "b1": b1, "W2": W2, "b2": b2, "W3": W3,
    })
    runner = _get_runner()
    outs = runner.run(per_core)
    dro_flat = outs[runner.out_names.index("dro")]
    dro_flat = np.asarray(dro_flat).reshape(N_CORES, N_LOC * M)
    dro = np.empty((B, N, M), np.float32)
    for c in range(N_CORES):
        b = c // 4
        n0 = (c % 4) * N_LOC
        dro[b, n0:n0 + N_LOC, :] = dro_flat[c].reshape(N_LOC, M)
    return dro + b3.reshape(-1)[0]


# revision 21
# speedup vs baseline: 1.0845x; 1.0845x over previous
"""Trainium2 Bass kernel for the pairwise-MLP GNN message-passing problem.

dro[b,n,m] = W3 . relu(W2^T relu(PhiA[b,n] @ W1a + PhiB[b,m] @ W1b + b1) + b2) + b3

Shapes (hardcoded): B=2, N=1024, M=256, D=576 (padded to 640), H1=512, H2=256.
Sharding: 8 cores over (B, N): core c handles b = c//4, n in [256*(c%4), 256*(c%4)+256).
Weights replicated. Each core computes its (256, 256) tile of dro independently.

Two kernel layouts, chosen at runtime:

v4 (used when b2 == 0, which holds for this problem's inputs):
- z2 computed TRANSPOSED: for each (m, n-block-of-128), stationary = h1 block
  [128 h1-slice, 128 n], moving = W2s [128, 256 h2] where W2s has |w3| folded
  into its columns (relu(c*x) = c*relu(x)) and columns sorted by sign(w3).
- layer 3 then is a signed reduce along the FREE dim of z2T [128 n, 256 h2]:
  DVE scalar_tensor_tensor (relu * sign, accum_out) for ~30% of blocks,
  ACT activation(Relu, accum_out) x2 (positive/negative column ranges) for the
  rest.  The PE never runs layer 3: PE work = layer2 (218us) + layer1 (4us).
- dro columns assemble naturally into [128 n, 256 m] SBUF tiles -> 2 DMAs.

e1a (fallback for arbitrary b2):
- h1 = relu(bplus + a_col) on DVE in bf16, layer 2 straight (h2 on partitions,
  pairs on free), relu2 = relu(z2+b2) via ACT bias, layer 3 on the PE.
"""

import os
import numpy as np
import ml_dtypes

B, N, M = 2, 1024, 256
D, D_PAD = 576, 640
H1, H2 = 512, 256
N_CORES = 8
N_LOC = N // 4          # 256 rows of dro per core
KT1 = D_PAD // 128      # 5 contraction tiles for layer 1
KT2 = H1 // 128         # 4 contraction tiles for layer 2
H1T = H1 // 128         # 4 partition tiles of h1
H2T = H2 // 128         # 2 partition tiles of h2
NB = N_LOC // 128       # 2 n-blocks of 128 (v4)
CHUNK_N = 2             # robot points per inner chunk (e1a)
PAIRS = CHUNK_N * M     # 512
N_CHUNKS = N_LOC // CHUNK_N   # 128

_RUNNER = None
_PATH = None            # 'v4' or 'e1a', set by build_per_core
_POS_CNT = None         # number of w3>=0 columns after the sign sort (v4)


def _round_fp32r(a):
    b = np.ascontiguousarray(a, dtype=np.float32).view(np.uint32)
    return ((b + np.uint32(0x800)) & np.uint32(0xFFFFF000)).view(np.float32)


def _to_bf16(a):
    return np.asarray(a, dtype=np.float32).astype(ml_dtypes.bfloat16)


def _split_multiwaits(bir_json):
    """This container's walrus accepts only one sync-wait command per
    instruction; hoist all but the last wait onto preceding same-engine
    EventSemaphore instructions (semantically identical: consecutive waits)."""
    import orjson

    d = orjson.loads(bir_json)
    for fn in d.get("functions", []):
        for blk in fn.get("blocks", []):
            insts = blk.get("instructions") or []
            out = []
            for inst in insts:
                si = inst.get("sync_info")
                waits = (si or {}).get("on_wait") or []
                if len(waits) > 1:
                    for j, w in enumerate(waits[:-1]):
                        out.append({
                            "debug": inst.get("debug", 0),
                            "engine": inst["engine"],
                            "ins": [],
                            "name": f"{inst['name']}-mw{j}",
                            "opcode": "EventSemaphore",
                            "outs": [],
                            "sync_info": {"on_update": [], "on_wait": [w]},
                        })
                    si["on_wait"] = [waits[-1]]
                out.append(inst)
            blk["instructions"] = out
    return orjson.dumps(d)


def _install_birfix():
    import concourse.bass2jax as b2j

    if getattr(b2j, "_multiwait_patched", False):
        return
    orig = b2j.compile_bir_kernel

    def patched(bir_json, tmpdir, neff_name="file.neff"):
        return orig(_split_multiwaits(bir_json), tmpdir, neff_name=neff_name)

    b2j.compile_bir_kernel = patched
    b2j._multiwait_patched = True


def _build_nc(repeat=1):
    if _PATH == "v4":
        return _build_nc_v4(repeat=repeat, pos=_POS_CNT)
    return _build_nc_e1a(repeat=repeat)


def _build_nc_v4(repeat=1, pos=128):
    import concourse.bass as bass
    import concourse.tile as tile
    import concourse.mybir as mybir

    f32 = mybir.dt.float32
    f32r = mybir.dt.float32r
    bf16 = mybir.dt.bfloat16
    add_op = mybir.AluOpType.add
    max_op = mybir.AluOpType.max
    mult_op = mybir.AluOpType.mult
    sub_op = mybir.AluOpType.subtract
    relu_fn = mybir.ActivationFunctionType.Relu

    nc = bass.Bass("TRN2", target_bir_lowering=False, debug=False,
                   num_devices=N_CORES)

    pa_ext = nc.dram_tensor("pa_t", [D_PAD, N_LOC], f32r, kind="ExternalInput")
    pb_ext = nc.dram_tensor("pb_t", [D_PAD, M], f32r, kind="ExternalInput")
    w1a_ext = nc.dram_tensor("w1a", [D_PAD, H1], f32r, kind="ExternalInput")
    w1b_ext = nc.dram_tensor("w1b", [D_PAD, H1], f32r, kind="ExternalInput")
    w2s_ext = nc.dram_tensor("w2s", [H1, H2], bf16, kind="ExternalInput")
    sgn_ext = nc.dram_tensor("sgn", [128, H2], f32, kind="ExternalInput")
    b1_ext = nc.dram_tensor("b1c", [H1, 1], f32, kind="ExternalInput")
    dro_ext = nc.dram_tensor("dro", [NB, 128, M], f32, kind="ExternalOutput")

    with tile.TileContext(nc) as tc:
        with tc.tile_pool(name="consts", bufs=1) as consts, \
             tc.tile_pool(name="proj", bufs=1) as proj:

            pa_sb, pb_sb, w1a_sb, w1b_sb = [], [], [], []
            for kt in range(KT1):
                t = consts.tile([128, N_LOC], f32r, tag=f"pa{kt}")
                nc.sync.dma_start(out=t, in_=pa_ext[kt * 128:(kt + 1) * 128, :])
                pa_sb.append(t)
                t = consts.tile([128, M], f32r, tag=f"pb{kt}")
                nc.sync.dma_start(out=t, in_=pb_ext[kt * 128:(kt + 1) * 128, :])
                pb_sb.append(t)
                t = consts.tile([128, H1], f32r, tag=f"w1a{kt}")
                nc.sync.dma_start(out=t, in_=w1a_ext[kt * 128:(kt + 1) * 128, :])
                w1a_sb.append(t)
                t = consts.tile([128, H1], f32r, tag=f"w1b{kt}")
                nc.sync.dma_start(out=t, in_=w1b_ext[kt * 128:(kt + 1) * 128, :])
                w1b_sb.append(t)
            w2s_sb = []
            for kt in range(KT2):
                t = consts.tile([128, H2], bf16, tag=f"w2s{kt}")
                nc.sync.dma_start(out=t, in_=w2s_ext[kt * 128:(kt + 1) * 128, :])
                w2s_sb.append(t)
            sgn_sb = consts.tile([128, H2], f32, tag="sgn")
            nc.sync.dma_start(out=sgn_sb, in_=sgn_ext[:, :])
            b1_sb = []
            for ht in range(H1T):
                t = consts.tile([128, 1], f32, tag=f"b1{ht}")
                nc.sync.dma_start(out=t, in_=b1_ext[ht * 128:(ht + 1) * 128, :])
                b1_sb.append(t)

            # persistent dro accumulators: dpos gets the positive-range sums
            # (or the full signed sum for DVE-handled blocks), dneg the
            # negative-range sums (stays 0 for DVE-handled columns)
            dpos_sb, dneg_sb, dout_sb = [], [], []
            for nb in range(NB):
                t = proj.tile([128, M], f32, tag=f"dp{nb}", name=f"dp{nb}")
                dpos_sb.append(t)
                t = proj.tile([128, M], f32, tag=f"dn{nb}", name=f"dn{nb}")
                nc.vector.memset(t, 0.0)
                dneg_sb.append(t)
                t = proj.tile([128, M], f32, tag=f"do{nb}", name=f"do{nb}")
                dout_sb.append(t)

            # ---- stage A: a_proj (bf16, streamed by DVE) and
            # bplus = b_proj + b1 (f32: tensor_scalar scalar operand) ----
            a_proj = []
            bplus = []
            with tc.tile_pool(name="apsum", bufs=2, space="PSUM") as apsum:
                for ht in range(H1T):
                    ps = apsum.tile([128, N_LOC], f32, tag="ps")
                    for kt in range(KT1):
                        nc.tensor.matmul(
                            ps, w1a_sb[kt][:, ht * 128:(ht + 1) * 128],
                            pa_sb[kt], start=(kt == 0), stop=(kt == KT1 - 1))
                    t = proj.tile([128, N_LOC], bf16, tag=f"ap{ht}")
                    nc.scalar.copy(t, ps)
                    a_proj.append(t)
                for ht in range(H1T):
                    ps = apsum.tile([128, M], f32, tag="ps")
                    for kt in range(KT1):
                        nc.tensor.matmul(
                            ps, w1b_sb[kt][:, ht * 128:(ht + 1) * 128],
                            pb_sb[kt], start=(kt == 0), stop=(kt == KT1 - 1))
                    t = proj.tile([128, M], f32, tag=f"bp{ht}")
                    nc.scalar.activation(
                        t, ps, mybir.ActivationFunctionType.Identity,
                        bias=b1_sb[ht])
                    bplus.append(t)

            # ---- stage B: loop over m; per m one h1 tile [h1, 256 n] and
            # two z2T psum tiles [128 n, 256 h2] ----
            with tc.tile_pool(name="hpool", bufs=3) as hpool, \
                 tc.tile_pool(name="scr", bufs=4) as scr, \
                 tc.tile_pool(name="zpsum", bufs=3, space="PSUM") as zpsum:
                for m_rep in range(repeat * M):
                    m = m_rep % M
                    # h1_m[k, n] = relu(a_proj[k, n] + bplus[k, m])  (bf16)
                    h1 = []
                    for kt in range(KT2):
                        t = hpool.tile([128, N_LOC], bf16, tag=f"h1_{kt}")
                        nc.vector.tensor_scalar(
                            out=t, in0=a_proj[kt],
                            scalar1=bplus[kt][:, m:m + 1],
                            scalar2=0.0, op0=add_op, op1=max_op)
                        h1.append(t)
                    for nb in range(NB):
                        zps = zpsum.tile([128, H2], f32, tag=f"z{nb}")
                        for kt in range(KT2):
                            nc.tensor.matmul(
                                zps, h1[kt][:, nb * 128:(nb + 1) * 128],
                                w2s_sb[kt], start=(kt == 0),
                                stop=(kt == KT2 - 1))
                        # signed reduce of relu(z2T) over h2 -> dro column.
                        # Two flavors, mixed 1:1 so ACT and DVE both land at
                        # ~235us next to the PE's 224us:
                        #  A: ACT relu+accum over pos/neg column ranges
                        #  C: DVE signed-reduce straight from PSUM (1x)
                        kind = (m * NB + nb) % 16
                        if kind < 8:
                            s = scr.tile([128, H2], bf16, tag="s")
                            if pos > 0:
                                nc.scalar.activation(
                                    s[:, 0:pos], zps[:, 0:pos], relu_fn,
                                    accum_out=dpos_sb[nb][:, m:m + 1])
                            else:
                                nc.vector.memset(dpos_sb[nb][:, m:m + 1], 0.0)
                            if pos < H2:
                                nc.scalar.activation(
                                    s[:, pos:H2], zps[:, pos:H2], relu_fn,
                                    accum_out=dneg_sb[nb][:, m:m + 1])
                        else:
                            s = scr.tile([128, H2], bf16, tag="s")
                            nc.vector.scalar_tensor_tensor(
                                out=s, in0=zps, scalar=0.0, in1=sgn_sb,
                                op0=max_op, op1=mult_op,
                                accum_out=dpos_sb[nb][:, m:m + 1])
                    if m == M - 1:
                        for nb in range(NB):
                            nc.vector.tensor_tensor(
                                out=dout_sb[nb], in0=dpos_sb[nb],
                                in1=dneg_sb[nb], op=sub_op)
                            nc.sync.dma_start(out=dro_ext[nb],
                                              in_=dout_sb[nb])
    return nc


def _build_nc_e1a(repeat=1):
    import concourse.bass as bass
    import concourse.tile as tile
    import concourse.mybir as mybir

    f32 = mybir.dt.float32
    f32r = mybir.dt.float32r
    bf16 = mybir.dt.bfloat16
    add_op = mybir.AluOpType.add
    max_op = mybir.AluOpType.max

    nc = bass.Bass("TRN2", target_bir_lowering=False, debug=False,
                   num_devices=N_CORES)

    pa_ext = nc.dram_tensor("pa_t", [D_PAD, N_LOC], f32r, kind="ExternalInput")
    pb_ext = nc.dram_tensor("pb_t", [D_PAD, M], f32r, kind="ExternalInput")
    w1a_ext = nc.dram_tensor("w1a", [D_PAD, H1], f32r, kind="ExternalInput")
    w1b_ext = nc.dram_tensor("w1b", [D_PAD, H1], f32r, kind="ExternalInput")
    w2_ext = nc.dram_tensor("w2", [H1, H2], bf16, kind="ExternalInput")
    w3_ext = nc.dram_tensor("w3", [H2, 1], bf16, kind="ExternalInput")
    b1_ext = nc.dram_tensor("b1c", [H1, 1], f32, kind="ExternalInput")
    b2_ext = nc.dram_tensor("b2c", [H2, 1], f32, kind="ExternalInput")
    dro_ext = nc.dram_tensor("dro", [1, N_LOC * M], f32,
                             kind="ExternalOutput")

    with tile.TileContext(nc) as tc:
        with tc.tile_pool(name="consts", bufs=1) as consts, \
             tc.tile_pool(name="proj", bufs=1) as proj:

            # ---- load constants ----
            pa_sb = []
            pb_sb = []
            w1a_sb = []
            w1b_sb = []
            for kt in range(KT1):
                t = consts.tile([128, N_LOC], f32r, tag=f"pa{kt}")
                nc.sync.dma_start(out=t, in_=pa_ext[kt * 128:(kt + 1) * 128, :])
                pa_sb.append(t)
                t = consts.tile([128, M], f32r, tag=f"pb{kt}")
                nc.sync.dma_start(out=t, in_=pb_ext[kt * 128:(kt + 1) * 128, :])
                pb_sb.append(t)
                t = consts.tile([128, H1], f32r, tag=f"w1a{kt}")
                nc.sync.dma_start(out=t, in_=w1a_ext[kt * 128:(kt + 1) * 128, :])
                w1a_sb.append(t)
                t = consts.tile([128, H1], f32r, tag=f"w1b{kt}")
                nc.sync.dma_start(out=t, in_=w1b_ext[kt * 128:(kt + 1) * 128, :])
                w1b_sb.append(t)
            w2_sb = []
            for kt in range(KT2):
                t = consts.tile([128, H2], bf16, tag=f"w2{kt}")
                nc.sync.dma_start(out=t, in_=w2_ext[kt * 128:(kt + 1) * 128, :])
                w2_sb.append(t)
            b1_sb = []
            for ht in range(H1T):
                t = consts.tile([128, 1], f32, tag=f"b1{ht}")
                nc.sync.dma_start(out=t, in_=b1_ext[ht * 128:(ht + 1) * 128, :])
                b1_sb.append(t)
            b2_sb = []
            w3_sb = []
            for ht in range(H2T):
                t = consts.tile([128, 1], f32, tag=f"b2{ht}")
                nc.sync.dma_start(out=t, in_=b2_ext[ht * 128:(ht + 1) * 128, :])
                b2_sb.append(t)
                t = consts.tile([128, 1], bf16, tag=f"w3{ht}")
                nc.sync.dma_start(out=t, in_=w3_ext[ht * 128:(ht + 1) * 128, :])
                w3_sb.append(t)

            # ---- stage A ----
            a_proj = []
            bplus = []
            with tc.tile_pool(name="apsum", bufs=2, space="PSUM") as apsum:
                for ht in range(H1T):
                    ps = apsum.tile([128, N_LOC], f32, tag="ps")
                    for kt in range(KT1):
                        nc.tensor.matmul(
                            ps, w1a_sb[kt][:, ht * 128:(ht + 1) * 128],
                            pa_sb[kt], start=(kt == 0), stop=(kt == KT1 - 1))
                    t = proj.tile([128, N_LOC], f32, tag=f"ap{ht}")
                    nc.scalar.copy(t, ps)
                    a_proj.append(t)
                for ht in range(H1T):
                    ps = apsum.tile([128, M], f32, tag="ps")
                    for kt in range(KT1):
                        nc.tensor.matmul(
                            ps, w1b_sb[kt][:, ht * 128:(ht + 1) * 128],
                            pb_sb[kt], start=(kt == 0), stop=(kt == KT1 - 1))
                    t = proj.tile([128, M], bf16, tag=f"bp{ht}")
                    nc.scalar.activation(
                        t, ps, mybir.ActivationFunctionType.Identity,
                        bias=b1_sb[ht])
                    bplus.append(t)

            # ---- stage B: chunks processed in pairs so each W2 stationary
            # block is loaded once per TWO 512-col matmuls (halves the
            # serial LDWEIGHTS cost; ldw-opt is disabled in this walrus) ----
            with tc.tile_pool(name="hpool", bufs=4) as hpool, \
                 tc.tile_pool(name="rpool", bufs=4) as rpool, \
                 tc.tile_pool(name="spool", bufs=2) as spool, \
                 tc.tile_pool(name="zpsum", bufs=1, space="PSUM") as zpsum, \
                 tc.tile_pool(name="dpsum", bufs=2, space="PSUM") as dpsum:
                stg = None
                for g_rep in range(repeat * (N_CHUNKS // 2)):
                    g2 = g_rep % (N_CHUNKS // 2)
                    cs = (2 * g2, 2 * g2 + 1)
                    h1 = [[None] * KT2 for _ in range(2)]
                    for ci, c in enumerate(cs):
                        for kt in range(KT2):
                            t = hpool.tile([128, PAIRS], bf16,
                                           tag=f"h1_{ci}_{kt}", name="t")
                            for half in range(CHUNK_N):
                                n = CHUNK_N * c + half
                                nc.vector.tensor_scalar(
                                    out=t[:, half * M:(half + 1) * M],
                                    in0=bplus[kt],
                                    scalar1=a_proj[kt][:, n:n + 1],
                                    scalar2=0.0,
                                    op0=add_op, op1=max_op)
                            h1[ci][kt] = t
                    relu2 = [[None] * H2T for _ in range(2)]
                    zt = [[None] * H2T for _ in range(2)]
                    for ci in range(2):
                        for ht in range(H2T):
                            zt[ci][ht] = zpsum.tile(
                                [128, PAIRS], f32, tag=f"z{ci}{ht}", name="z")
                    for ht in range(H2T):
                        for kt in range(KT2):
                            for ci in range(2):
                                nc.tensor.matmul(
                                    zt[ci][ht],
                                    w2_sb[kt][:, ht * 128:(ht + 1) * 128],
                                    h1[ci][kt], start=(kt == 0),
                                    stop=(kt == KT2 - 1))
                        for ci in range(2):
                            r = rpool.tile([128, PAIRS], bf16,
                                           tag=f"r{ci}{ht}", name="r")
                            nc.scalar.activation(
                                r, zt[ci][ht],
                                mybir.ActivationFunctionType.Relu,
                                bias=b2_sb[ht])
                            relu2[ci][ht] = r
                    for ci, c in enumerate(cs):
                        dps = dpsum.tile([1, PAIRS], f32, tag="d", name="d")
                        for ht in range(H2T):
                            nc.tensor.matmul(dps, w3_sb[ht], relu2[ci][ht],
                                             start=(ht == 0),
                                             stop=(ht == H2T - 1))
                        s = c % 8
                        if s == 0:
                            stg = spool.tile([1, 8 * PAIRS], f32, tag="stg",
                                             name="stg")
                        if c % 2 == 0:
                            nc.scalar.copy(stg[:, s * PAIRS:(s + 1) * PAIRS],
                                           dps)
                        else:
                            nc.vector.tensor_copy(
                                out=stg[:, s * PAIRS:(s + 1) * PAIRS], in_=dps)
                        if s == 7:
                            g = c // 8
                            sz = 8 * PAIRS
                            nc.sync.dma_start(
                                out=dro_ext[:, g * sz:(g + 1) * sz], in_=stg)
    return nc


class _Runner:
    def __init__(self, repeat=1):
        _install_birfix()
        import jax
        import numpy as _np
        from jax.sharding import Mesh, PartitionSpec
        from jax.experimental.shard_map import shard_map
        import concourse.bass2jax as b2j
        import concourse.mybir as mybir

        nc = _build_nc(repeat=repeat)
        self.nc = nc
        b2j.install_neuronx_cc_hook()

        partition_name = (nc.partition_id_tensor.name
                          if nc.partition_id_tensor else None)
        in_names, out_names, out_avals, zero_outs = [], [], [], []
        for alloc in nc.m.functions[0].allocations:
            if not isinstance(alloc, mybir.MemoryLocationSet):
                continue
            name = alloc.memorylocations[0].name
            if alloc.kind == "ExternalInput":
                if name != partition_name:
                    in_names.append(name)
            elif alloc.kind == "ExternalOutput":
                shape = tuple(alloc.tensor_shape)
                dtype = mybir.dt.np(alloc.dtype)
                out_names.append(name)
                out_avals.append(jax.core.ShapedArray(shape, dtype))
                zero_outs.append(_np.zeros(shape, dtype))
        n_params = len(in_names)
        self.in_names = list(in_names)
        self.out_names = out_names
        self.zero_outs = zero_outs
        bind_names = list(in_names) + list(out_names)
        if partition_name is not None:
            bind_names.append(partition_name)

        def _body(*args):
            operands = list(args)
            if partition_name is not None:
                operands.append(b2j.partition_id_tensor())
            outs = b2j._bass_exec_p.bind(
                *operands,
                out_avals=tuple(out_avals),
                in_names=tuple(bind_names),
                out_names=tuple(out_names),
                lowering_input_output_aliases=(),
                sim_require_finite=True,
                sim_require_nnan=True,
                nc=nc,
            )
            return tuple(outs)

        devices = jax.devices()[:N_CORES]
        assert len(devices) == N_CORES, f"need {N_CORES} cores, have {devices}"
        mesh = Mesh(_np.asarray(devices), ("core",))
        n_outs = len(out_names)
        self.fn = jax.jit(
            shard_map(_body, mesh=mesh,
                      in_specs=(PartitionSpec("core"),) * (n_params + n_outs),
                      out_specs=(PartitionSpec("core"),) * n_outs,
                      check_rep=False),
            keep_unused=True,
        )
        self.jax = jax

    def run(self, per_core_maps):
        np_ = np
        concat_in = [
            np_.concatenate([m[name] for m in per_core_maps], axis=0)
            for name in self.in_names
        ]
        concat_zero = [
            np_.zeros((N_CORES * z.shape[0], *z.shape[1:]), z.dtype)
            for z in self.zero_outs
        ]
        out = self.fn(*concat_in, *concat_zero)
        out = [np_.asarray(o) for o in out]
        return out

    def time_ns(self, per_core_maps, iters=10):
        import time
        jax = self.jax
        concat_in = [
            np.concatenate([m[name] for m in per_core_maps], axis=0)
            for name in self.in_names
        ]
        concat_zero = [
            np.zeros((N_CORES * z.shape[0], *z.shape[1:]), z.dtype)
            for z in self.zero_outs
        ]
        dev_in = [jax.device_put(a) for a in concat_in]
        dev_zero = [jax.device_put(a) for a in concat_zero]
        r = self.fn(*dev_in, *dev_zero)
        jax.block_until_ready(r)
        best = float("inf")
        for _ in range(iters):
            t0 = time.perf_counter_ns()
            r = self.fn(*dev_in, *dev_zero)
            jax.block_until_ready(r)
            dt = time.perf_counter_ns() - t0
            best = min(best, dt)
        return best


def _get_runner():
    global _RUNNER
    if _RUNNER is None:
        _RUNNER = _Runner()
    return _RUNNER


def build_per_core(inputs):
    """Shard + lay out the full inputs into per-core input maps.  Also decides
    which kernel layout to use (v4 requires b2 == 0) and sets _PATH/_POS_CNT."""
    global _PATH, _POS_CNT
    Phi_A = np.asarray(inputs["Phi_A"], dtype=np.float32)
    Phi_B = np.asarray(inputs["Phi_B"], dtype=np.float32)
    W1a = np.asarray(inputs["W1a"], dtype=np.float32)
    W1b = np.asarray(inputs["W1b"], dtype=np.float32)
    W2 = np.asarray(inputs["W2"], dtype=np.float32)
    W3 = np.asarray(inputs["W3"], dtype=np.float32)
    b1 = np.asarray(inputs["b1"], dtype=np.float32)
    b2 = np.asarray(inputs["b2"], dtype=np.float32)

    path = "e1a"  # v4 (transposed) loses on HW: per-matmul stationary reloads
    if _PATH is None:
        _PATH = path
    else:
        assert _PATH == path, "kernel layout fixed after first call"

    w1a_p = np.zeros((D_PAD, H1), np.float32)
    w1a_p[:D] = W1a
    w1b_p = np.zeros((D_PAD, H1), np.float32)
    w1b_p[:D] = W1b
    w1a_p = _round_fp32r(w1a_p)
    w1b_p = _round_fp32r(w1b_p)
    b1c = b1.reshape(H1, 1)

    com = {
        "w1a": w1a_p,
        "w1b": w1b_p,
        "b1c": b1c,
    }
    w3f = W3.reshape(H2)
    if path == "v4":
        # fold |w3| into W2 columns (relu(c*x) = c*relu(x), c >= 0), sort
        # columns so w3 >= 0 comes first; sign row handles the subtraction
        perm = np.argsort(w3f < 0, kind="stable")
        pos = int((w3f >= 0).sum())
        w2s = (W2 * np.abs(w3f)[None, :])[:, perm]
        sgn = np.sign(w3f)[perm]
        com["w2s"] = _to_bf16(w2s)
        com["sgn"] = np.ascontiguousarray(
            np.broadcast_to(sgn, (128, H2)), dtype=np.float32)
        _POS_CNT = pos
    else:
        com["w2"] = _to_bf16(W2)
        com["w3"] = _to_bf16(W3.reshape(H2, 1))
        com["b2c"] = b2.reshape(H2, 1)

    per_core = []
    for c in range(N_CORES):
        b = c // 4
        n0 = (c % 4) * N_LOC
        pa = np.zeros((D_PAD, N_LOC), np.float32)
        pa[:D] = Phi_A[b, n0:n0 + N_LOC, :].T
        pb = np.zeros((D_PAD, M), np.float32)
        pb[:D] = Phi_B[b].T
        per_core.append(dict(com, pa_t=_round_fp32r(pa), pb_t=_round_fp32r(pb)))
    return per_core


def kernel(Phi_A, Phi_B, W1a, W1b, b1, W2, b2, W3, b3):
    b3 = np.asarray(b3, dtype=np.float32)
    per_core = build_per_core({
        "Phi_A": Phi_A, "Phi_B": Phi_B, "W1a": W1a, "W1b": W1b,
        "b1": b1, "W2": W2, "b2": b2, "W3": W3,
    })
    runner = _get_runner()
    outs = runner.run(per_core)
    dro_flat = outs[runner.out_names.index("dro")]
    dro_flat = np.asarray(dro_flat).reshape(N_CORES, N_LOC * M)
    dro = np.empty((B, N, M), np.float32)
    for c in range(N_CORES):
        b = c // 4
        n0 = (c % 4) * N_LOC
        dro[b, n0:n0 + N_LOC, :] = dro_flat[c].reshape(N_LOC, M)
    return dro + b3.reshape(-1)[0]


# revision 23
# speedup vs baseline: 1.1112x; 1.0245x over previous
"""Trainium2 Bass kernel for the pairwise-MLP GNN message-passing problem.

dro[b,n,m] = W3 . relu(W2^T relu(PhiA[b,n] @ W1a + PhiB[b,m] @ W1b + b1) + b2) + b3

Shapes (hardcoded): B=2, N=1024, M=256, D=576 (padded to 640), H1=512, H2=256.
Sharding: 8 cores over (B, N): core c handles b = c//4, n in [256*(c%4), 256*(c%4)+256).
Weights replicated. Each core computes its (256, 256) tile of dro independently.

Two kernel layouts, chosen at runtime:

v4 (used when b2 == 0, which holds for this problem's inputs):
- z2 computed TRANSPOSED: for each (m, n-block-of-128), stationary = h1 block
  [128 h1-slice, 128 n], moving = W2s [128, 256 h2] where W2s has |w3| folded
  into its columns (relu(c*x) = c*relu(x)) and columns sorted by sign(w3).
- layer 3 then is a signed reduce along the FREE dim of z2T [128 n, 256 h2]:
  DVE scalar_tensor_tensor (relu * sign, accum_out) for ~30% of blocks,
  ACT activation(Relu, accum_out) x2 (positive/negative column ranges) for the
  rest.  The PE never runs layer 3: PE work = layer2 (218us) + layer1 (4us).
- dro columns assemble naturally into [128 n, 256 m] SBUF tiles -> 2 DMAs.

e1a (fallback for arbitrary b2):
- h1 = relu(bplus + a_col) on DVE in bf16, layer 2 straight (h2 on partitions,
  pairs on free), relu2 = relu(z2+b2) via ACT bias, layer 3 on the PE.
"""

import os
import numpy as np
import ml_dtypes

B, N, M = 2, 1024, 256
D, D_PAD = 576, 640
H1, H2 = 512, 256
N_CORES = 8
N_LOC = N // 4          # 256 rows of dro per core
KT1 = D_PAD // 128      # 5 contraction tiles for layer 1
KT2 = H1 // 128         # 4 contraction tiles for layer 2
H1T = H1 // 128         # 4 partition tiles of h1
H2T = H2 // 128         # 2 partition tiles of h2
NB = N_LOC // 128       # 2 n-blocks of 128 (v4)
CHUNK_N = 2             # robot points per inner chunk (e1a)
PAIRS = CHUNK_N * M     # 512
N_CHUNKS = N_LOC // CHUNK_N   # 128

_RUNNER = None
_PATH = None            # 'v4' or 'e1a', set by build_per_core
_POS_CNT = None         # number of w3>=0 columns after the sign sort (v4)


def _round_fp32r(a):
    b = np.ascontiguousarray(a, dtype=np.float32).view(np.uint32)
    return ((b + np.uint32(0x800)) & np.uint32(0xFFFFF000)).view(np.float32)


def _to_bf16(a):
    return np.asarray(a, dtype=np.float32).astype(ml_dtypes.bfloat16)


def _split_multiwaits(bir_json):
    """This container's walrus accepts only one sync-wait command per
    instruction; hoist all but the last wait onto preceding same-engine
    EventSemaphore instructions (semantically identical: consecutive waits)."""
    import orjson

    d = orjson.loads(bir_json)
    for fn in d.get("functions", []):
        for blk in fn.get("blocks", []):
            insts = blk.get("instructions") or []
            out = []
            for inst in insts:
                si = inst.get("sync_info")
                waits = (si or {}).get("on_wait") or []
                if len(waits) > 1:
                    for j, w in enumerate(waits[:-1]):
                        out.append({
                            "debug": inst.get("debug", 0),
                            "engine": inst["engine"],
                            "ins": [],
                            "name": f"{inst['name']}-mw{j}",
                            "opcode": "EventSemaphore",
                            "outs": [],
                            "sync_info": {"on_update": [], "on_wait": [w]},
                        })
                    si["on_wait"] = [waits[-1]]
                out.append(inst)
            blk["instructions"] = out
    return orjson.dumps(d)


def _install_birfix():
    import concourse.bass2jax as b2j

    if getattr(b2j, "_multiwait_patched", False):
        return
    orig = b2j.compile_bir_kernel

    def patched(bir_json, tmpdir, neff_name="file.neff"):
        return orig(_split_multiwaits(bir_json), tmpdir, neff_name=neff_name)

    b2j.compile_bir_kernel = patched
    b2j._multiwait_patched = True


def _build_nc(repeat=1):
    if _PATH == "v4":
        return _build_nc_v4(repeat=repeat, pos=_POS_CNT)
    return _build_nc_e1a(repeat=repeat)


def _build_nc_v4(repeat=1, pos=128):
    import concourse.bass as bass
    import concourse.tile as tile
    import concourse.mybir as mybir

    f32 = mybir.dt.float32
    f32r = mybir.dt.float32r
    bf16 = mybir.dt.bfloat16
    add_op = mybir.AluOpType.add
    max_op = mybir.AluOpType.max
    mult_op = mybir.AluOpType.mult
    sub_op = mybir.AluOpType.subtract
    relu_fn = mybir.ActivationFunctionType.Relu

    nc = bass.Bass("TRN2", target_bir_lowering=False, debug=False,
                   num_devices=N_CORES)

    pa_ext = nc.dram_tensor("pa_t", [D_PAD, N_LOC], f32r, kind="ExternalInput")
    pb_ext = nc.dram_tensor("pb_t", [D_PAD, M], f32r, kind="ExternalInput")
    w1a_ext = nc.dram_tensor("w1a", [D_PAD, H1], f32r, kind="ExternalInput")
    w1b_ext = nc.dram_tensor("w1b", [D_PAD, H1], f32r, kind="ExternalInput")
    w2s_ext = nc.dram_tensor("w2s", [H1, H2], bf16, kind="ExternalInput")
    sgn_ext = nc.dram_tensor("sgn", [128, H2], f32, kind="ExternalInput")
    b1_ext = nc.dram_tensor("b1c", [H1, 1], f32, kind="ExternalInput")
    dro_ext = nc.dram_tensor("dro", [NB, 128, M], f32, kind="ExternalOutput")

    with tile.TileContext(nc) as tc:
        with tc.tile_pool(name="consts", bufs=1) as consts, \
             tc.tile_pool(name="proj", bufs=1) as proj:

            pa_sb, pb_sb, w1a_sb, w1b_sb = [], [], [], []
            for kt in range(KT1):
                t = consts.tile([128, N_LOC], f32r, tag=f"pa{kt}")
                nc.sync.dma_start(out=t, in_=pa_ext[kt * 128:(kt + 1) * 128, :])
                pa_sb.append(t)
                t = consts.tile([128, M], f32r, tag=f"pb{kt}")
                nc.sync.dma_start(out=t, in_=pb_ext[kt * 128:(kt + 1) * 128, :])
                pb_sb.append(t)
                t = consts.tile([128, H1], f32r, tag=f"w1a{kt}")
                nc.sync.dma_start(out=t, in_=w1a_ext[kt * 128:(kt + 1) * 128, :])
                w1a_sb.append(t)
                t = consts.tile([128, H1], f32r, tag=f"w1b{kt}")
                nc.sync.dma_start(out=t, in_=w1b_ext[kt * 128:(kt + 1) * 128, :])
                w1b_sb.append(t)
            w2s_sb = []
            for kt in range(KT2):
                t = consts.tile([128, H2], bf16, tag=f"w2s{kt}")
                nc.sync.dma_start(out=t, in_=w2s_ext[kt * 128:(kt + 1) * 128, :])
                w2s_sb.append(t)
            sgn_sb = consts.tile([128, H2], f32, tag="sgn")
            nc.sync.dma_start(out=sgn_sb, in_=sgn_ext[:, :])
            b1_sb = []
            for ht in range(H1T):
                t = consts.tile([128, 1], f32, tag=f"b1{ht}")
                nc.sync.dma_start(out=t, in_=b1_ext[ht * 128:(ht + 1) * 128, :])
                b1_sb.append(t)

            # persistent dro accumulators: dpos gets the positive-range sums
            # (or the full signed sum for DVE-handled blocks), dneg the
            # negative-range sums (stays 0 for DVE-handled columns)
            dpos_sb, dneg_sb, dout_sb = [], [], []
            for nb in range(NB):
                t = proj.tile([128, M], f32, tag=f"dp{nb}", name=f"dp{nb}")
                dpos_sb.append(t)
                t = proj.tile([128, M], f32, tag=f"dn{nb}", name=f"dn{nb}")
                nc.vector.memset(t, 0.0)
                dneg_sb.append(t)
                t = proj.tile([128, M], f32, tag=f"do{nb}", name=f"do{nb}")
                dout_sb.append(t)

            # ---- stage A: a_proj (bf16, streamed by DVE) and
            # bplus = b_proj + b1 (f32: tensor_scalar scalar operand) ----
            a_proj = []
            bplus = []
            with tc.tile_pool(name="apsum", bufs=2, space="PSUM") as apsum:
                for ht in range(H1T):
                    ps = apsum.tile([128, N_LOC], f32, tag="ps")
                    for kt in range(KT1):
                        nc.tensor.matmul(
                            ps, w1a_sb[kt][:, ht * 128:(ht + 1) * 128],
                            pa_sb[kt], start=(kt == 0), stop=(kt == KT1 - 1))
                    t = proj.tile([128, N_LOC], bf16, tag=f"ap{ht}")
                    nc.scalar.copy(t, ps)
                    a_proj.append(t)
                for ht in range(H1T):
                    ps = apsum.tile([128, M], f32, tag="ps")
                    for kt in range(KT1):
                        nc.tensor.matmul(
                            ps, w1b_sb[kt][:, ht * 128:(ht + 1) * 128],
                            pb_sb[kt], start=(kt == 0), stop=(kt == KT1 - 1))
                    t = proj.tile([128, M], f32, tag=f"bp{ht}")
                    nc.scalar.activation(
                        t, ps, mybir.ActivationFunctionType.Identity,
                        bias=b1_sb[ht])
                    bplus.append(t)

            # ---- stage B: loop over m; per m one h1 tile [h1, 256 n] and
            # two z2T psum tiles [128 n, 256 h2] ----
            with tc.tile_pool(name="hpool", bufs=3) as hpool, \
                 tc.tile_pool(name="scr", bufs=4) as scr, \
                 tc.tile_pool(name="zpsum", bufs=3, space="PSUM") as zpsum:
                for m_rep in range(repeat * M):
                    m = m_rep % M
                    # h1_m[k, n] = relu(a_proj[k, n] + bplus[k, m])  (bf16)
                    h1 = []
                    for kt in range(KT2):
                        t = hpool.tile([128, N_LOC], bf16, tag=f"h1_{kt}")
                        nc.vector.tensor_scalar(
                            out=t, in0=a_proj[kt],
                            scalar1=bplus[kt][:, m:m + 1],
                            scalar2=0.0, op0=add_op, op1=max_op)
                        h1.append(t)
                    for nb in range(NB):
                        zps = zpsum.tile([128, H2], f32, tag=f"z{nb}")
                        for kt in range(KT2):
                            nc.tensor.matmul(
                                zps, h1[kt][:, nb * 128:(nb + 1) * 128],
                                w2s_sb[kt], start=(kt == 0),
                                stop=(kt == KT2 - 1))
                        # signed reduce of relu(z2T) over h2 -> dro column.
                        # Two flavors, mixed 1:1 so ACT and DVE both land at
                        # ~235us next to the PE's 224us:
                        #  A: ACT relu+accum over pos/neg column ranges
                        #  C: DVE signed-reduce straight from PSUM (1x)
                        kind = (m * NB + nb) % 16
                        if kind < 8:
                            s = scr.tile([128, H2], bf16, tag="s")
                            if pos > 0:
                                nc.scalar.activation(
                                    s[:, 0:pos], zps[:, 0:pos], relu_fn,
                                    accum_out=dpos_sb[nb][:, m:m + 1])
                            else:
                                nc.vector.memset(dpos_sb[nb][:, m:m + 1], 0.0)
                            if pos < H2:
                                nc.scalar.activation(
                                    s[:, pos:H2], zps[:, pos:H2], relu_fn,
                                    accum_out=dneg_sb[nb][:, m:m + 1])
                        else:
                            s = scr.tile([128, H2], bf16, tag="s")
                            nc.vector.scalar_tensor_tensor(
                                out=s, in0=zps, scalar=0.0, in1=sgn_sb,
                                op0=max_op, op1=mult_op,
                                accum_out=dpos_sb[nb][:, m:m + 1])
                    if m == M - 1:
                        for nb in range(NB):
                            nc.vector.tensor_tensor(
                                out=dout_sb[nb], in0=dpos_sb[nb],
                                in1=dneg_sb[nb], op=sub_op)
                            nc.sync.dma_start(out=dro_ext[nb],
                                              in_=dout_sb[nb])
    return nc


def _build_nc_e1a(repeat=1):
    import concourse.bass as bass
    import concourse.tile as tile
    import concourse.mybir as mybir

    f32 = mybir.dt.float32
    f32r = mybir.dt.float32r
    bf16 = mybir.dt.bfloat16
    add_op = mybir.AluOpType.add
    max_op = mybir.AluOpType.max

    nc = bass.Bass("TRN2", target_bir_lowering=False, debug=False,
                   num_devices=N_CORES)

    pa_ext = nc.dram_tensor("pa_t", [D_PAD, N_LOC], f32r, kind="ExternalInput")
    pb_ext = nc.dram_tensor("pb_t", [D_PAD, M], f32r, kind="ExternalInput")
    w1a_ext = nc.dram_tensor("w1a", [D_PAD, H1], f32r, kind="ExternalInput")
    w1b_ext = nc.dram_tensor("w1b", [D_PAD, H1], f32r, kind="ExternalInput")
    w2_ext = nc.dram_tensor("w2", [H1, H2], bf16, kind="ExternalInput")
    w3_ext = nc.dram_tensor("w3", [H2, 1], bf16, kind="ExternalInput")
    b1_ext = nc.dram_tensor("b1c", [H1, 1], f32, kind="ExternalInput")
    b2_ext = nc.dram_tensor("b2c", [H2, 1], f32, kind="ExternalInput")
    dro_ext = nc.dram_tensor("dro", [1, N_LOC * M], f32,
                             kind="ExternalOutput")

    with tile.TileContext(nc) as tc:
        with tc.tile_pool(name="consts", bufs=1) as consts, \
             tc.tile_pool(name="proj", bufs=1) as proj:

            # ---- load constants ----
            pa_sb = []
            pb_sb = []
            w1a_sb = []
            w1b_sb = []
            for kt in range(KT1):
                t = consts.tile([128, N_LOC], f32r, tag=f"pa{kt}")
                nc.sync.dma_start(out=t, in_=pa_ext[kt * 128:(kt + 1) * 128, :])
                pa_sb.append(t)
                t = consts.tile([128, M], f32r, tag=f"pb{kt}")
                nc.sync.dma_start(out=t, in_=pb_ext[kt * 128:(kt + 1) * 128, :])
                pb_sb.append(t)
                t = consts.tile([128, H1], f32r, tag=f"w1a{kt}")
                nc.sync.dma_start(out=t, in_=w1a_ext[kt * 128:(kt + 1) * 128, :])
                w1a_sb.append(t)
                t = consts.tile([128, H1], f32r, tag=f"w1b{kt}")
                nc.sync.dma_start(out=t, in_=w1b_ext[kt * 128:(kt + 1) * 128, :])
                w1b_sb.append(t)
            w2_sb = []
            for kt in range(KT2):
                t = consts.tile([128, H2], bf16, tag=f"w2{kt}")
                nc.sync.dma_start(out=t, in_=w2_ext[kt * 128:(kt + 1) * 128, :])
                w2_sb.append(t)
            b1_sb = []
            for ht in range(H1T):
                t = consts.tile([128, 1], f32, tag=f"b1{ht}")
                nc.sync.dma_start(out=t, in_=b1_ext[ht * 128:(ht + 1) * 128, :])
                b1_sb.append(t)
            b2_sb = []
            w3_sb = []
            for ht in range(H2T):
                t = consts.tile([128, 1], f32, tag=f"b2{ht}")
                nc.sync.dma_start(out=t, in_=b2_ext[ht * 128:(ht + 1) * 128, :])
                b2_sb.append(t)
                t = consts.tile([128, 1], bf16, tag=f"w3{ht}")
                nc.sync.dma_start(out=t, in_=w3_ext[ht * 128:(ht + 1) * 128, :])
                w3_sb.append(t)

            # ---- stage A ----
            a_proj = []
            bplus = []
            with tc.tile_pool(name="apsum", bufs=2, space="PSUM") as apsum:
                for ht in range(H1T):
                    ps = apsum.tile([128, N_LOC], f32, tag="ps")
                    for kt in range(KT1):
                        nc.tensor.matmul(
                            ps, w1a_sb[kt][:, ht * 128:(ht + 1) * 128],
                            pa_sb[kt], start=(kt == 0), stop=(kt == KT1 - 1))
                    t = proj.tile([128, N_LOC], f32, tag=f"ap{ht}")
                    nc.scalar.copy(t, ps)
                    a_proj.append(t)
                for ht in range(H1T):
                    ps = apsum.tile([128, M], f32, tag="ps")
                    for kt in range(KT1):
                        nc.tensor.matmul(
                            ps, w1b_sb[kt][:, ht * 128:(ht + 1) * 128],
                            pb_sb[kt], start=(kt == 0), stop=(kt == KT1 - 1))
                    t = proj.tile([128, M], bf16, tag=f"bp{ht}")
                    nc.scalar.activation(
                        t, ps, mybir.ActivationFunctionType.Identity,
                        bias=b1_sb[ht])
                    bplus.append(t)

            # ---- stage B: chunks processed in pairs so each W2 stationary
            # block is loaded once per TWO 512-col matmuls (halves the
            # serial LDWEIGHTS cost; ldw-opt is disabled in this walrus) ----
            with tc.tile_pool(name="hpool", bufs=4) as hpool, \
                 tc.tile_pool(name="rpool", bufs=4) as rpool, \
                 tc.tile_pool(name="spool", bufs=2) as spool, \
                 tc.tile_pool(name="zpsum", bufs=1, space="PSUM") as zpsum, \
                 tc.tile_pool(name="dpsum", bufs=2, space="PSUM") as dpsum:
                stg = None
                for g_rep in range(repeat * (N_CHUNKS // 2)):
                    g2 = g_rep % (N_CHUNKS // 2)
                    cs = (2 * g2, 2 * g2 + 1)
                    h1 = [[None] * KT2 for _ in range(2)]
                    for ci, c in enumerate(cs):
                        for kt in range(KT2):
                            t = hpool.tile([128, PAIRS], bf16,
                                           tag=f"h1_{ci}_{kt}", name="t")
                            for half in range(CHUNK_N):
                                n = CHUNK_N * c + half
                                nc.vector.tensor_scalar(
                                    out=t[:, half * M:(half + 1) * M],
                                    in0=bplus[kt],
                                    scalar1=a_proj[kt][:, n:n + 1],
                                    scalar2=0.0,
                                    op0=add_op, op1=max_op)
                            h1[ci][kt] = t
                    relu2 = [[None] * H2T for _ in range(2)]
                    zt = [[None] * H2T for _ in range(2)]
                    for ci in range(2):
                        for ht in range(H2T):
                            zt[ci][ht] = zpsum.tile(
                                [128, PAIRS], f32, tag=f"z{ci}{ht}", name="z")
                    for ht in range(H2T):
                        for kt in range(KT2):
                            for ci in range(2):
                                nc.tensor.matmul(
                                    zt[ci][ht],
                                    w2_sb[kt][:, ht * 128:(ht + 1) * 128],
                                    h1[ci][kt], start=(kt == 0),
                                    stop=(kt == KT2 - 1))
                        for ci in range(2):
                            r = rpool.tile([128, PAIRS], bf16,
                                           tag=f"r{ci}{ht}", name="r")
                            nc.scalar.activation(
                                r, zt[ci][ht],
                                mybir.ActivationFunctionType.Relu,
                                bias=b2_sb[ht])
                            relu2[ci][ht] = r
                    for ci, c in enumerate(cs):
                        dps = dpsum.tile([1, PAIRS], f32, tag="d", name="d")
                        for ht in range(H2T):
                            nc.tensor.matmul(dps, w3_sb[ht], relu2[ci][ht],
                                             start=(ht == 0),
                                             stop=(ht == H2T - 1))
                        s = c % 8
                        if s == 0:
                            stg = spool.tile([1, 8 * PAIRS], f32, tag="stg",
                                             name="stg")
                        if c % 2 == 0:
                            nc.scalar.copy(stg[:, s * PAIRS:(s + 1) * PAIRS],
                                           dps)
                        else:
                            nc.vector.tensor_copy(
                                out=stg[:, s * PAIRS:(s + 1) * PAIRS], in_=dps)
                        if s == 7:
                            g = c // 8
                            sz = 8 * PAIRS
                            nc.sync.dma_start(
                                out=dro_ext[:, g * sz:(g + 1) * sz], in_=stg)
    return nc


class _Runner:
    def __init__(self, repeat=1):
        _install_birfix()
        import jax
        import numpy as _np
        from jax.sharding import Mesh, PartitionSpec
        from jax.experimental.shard_map import shard_map
        import concourse.bass2jax as b2j
        import concourse.mybir as mybir

        nc = _build_nc(repeat=repeat)
        self.nc = nc
        b2j.install_neuronx_cc_hook()

        partition_name = (nc.partition_id_tensor.name
                          if nc.partition_id_tensor else None)
        in_names, out_names, out_avals, zero_outs = [], [], [], []
        for alloc in nc.m.functions[0].allocations:
            if not isinstance(alloc, mybir.MemoryLocationSet):
                continue
            name = alloc.memorylocations[0].name
            if alloc.kind == "ExternalInput":
                if name != partition_name:
                    in_names.append(name)
            elif alloc.kind == "ExternalOutput":
                shape = tuple(alloc.tensor_shape)
                dtype = mybir.dt.np(alloc.dtype)
                out_names.append(name)
                out_avals.append(jax.core.ShapedArray(shape, dtype))
                zero_outs.append(_np.zeros(shape, dtype))
        n_params = len(in_names)
        self.in_names = list(in_names)
        self.out_names = out_names
        self.zero_outs = zero_outs
        bind_names = list(in_names) + list(out_names)
        if partition_name is not None:
            bind_names.append(partition_name)

        def _body(*args):
            operands = list(args)
            if partition_name is not None:
                operands.append(b2j.partition_id_tensor())
            outs = b2j._bass_exec_p.bind(
                *operands,
                out_avals=tuple(out_avals),
                in_names=tuple(bind_names),
                out_names=tuple(out_names),
                lowering_input_output_aliases=(),
                sim_require_finite=True,
                sim_require_nnan=True,
                nc=nc,
            )
            return tuple(outs)

        devices = jax.devices()[:N_CORES]
        assert len(devices) == N_CORES, f"need {N_CORES} cores, have {devices}"
        mesh = Mesh(_np.asarray(devices), ("core",))
        n_outs = len(out_names)
        self.fn = jax.jit(
            shard_map(_body, mesh=mesh,
                      in_specs=(PartitionSpec("core"),) * (n_params + n_outs),
                      out_specs=(PartitionSpec("core"),) * n_outs,
                      check_rep=False),
            keep_unused=True,
        )
        self.jax = jax

    def run(self, per_core_maps):
        np_ = np
        concat_in = [
            np_.concatenate([m[name] for m in per_core_maps], axis=0)
            for name in self.in_names
        ]
        concat_zero = [
            np_.zeros((N_CORES * z.shape[0], *z.shape[1:]), z.dtype)
            for z in self.zero_outs
        ]
        out = self.fn(*concat_in, *concat_zero)
        out = [np_.asarray(o) for o in out]
        return out

    def time_ns(self, per_core_maps, iters=10):
        import time
        jax = self.jax
        concat_in = [
            np.concatenate([m[name] for m in per_core_maps], axis=0)
            for name in self.in_names
        ]
        concat_zero = [
            np.zeros((N_CORES * z.shape[0], *z.shape[1:]), z.dtype)
            for z in self.zero_outs
        ]
        dev_in = [jax.device_put(a) for a in concat_in]
        dev_zero = [jax.device_put(a) for a in concat_zero]
        r = self.fn(*dev_in, *dev_zero)
        jax.block_until_ready(r)
        best = float("inf")
        for _ in range(iters):
            t0 = time.perf_counter_ns()
            r = self.fn(*dev_in, *dev_zero)
            jax.block_until_ready(r)
            dt = time.perf_counter_ns() - t0
            best = min(best, dt)
        return best


def _get_runner():
    global _RUNNER
    if _RUNNER is None:
        _RUNNER = _Runner()
    return _RUNNER


def build_per_core(inputs):
    """Shard + lay out the full inputs into per-core input maps.  Also decides
    which kernel layout to use (v4 requires b2 == 0) and sets _PATH/_POS_CNT."""
    global _PATH, _POS_CNT
    Phi_A = np.asarray(inputs["Phi_A"], dtype=np.float32)
    Phi_B = np.asarray(inputs["Phi_B"], dtype=np.float32)
    W1a = np.asarray(inputs["W1a"], dtype=np.float32)
    W1b = np.asarray(inputs["W1b"], dtype=np.float32)
    W2 = np.asarray(inputs["W2"], dtype=np.float32)
    W3 = np.asarray(inputs["W3"], dtype=np.float32)
    b1 = np.asarray(inputs["b1"], dtype=np.float32)
    b2 = np.asarray(inputs["b2"], dtype=np.float32)

    path = "e1a"  # v4 (transposed) loses on HW: per-matmul stationary reloads
    if _PATH is None:
        _PATH = path
    else:
        assert _PATH == path, "kernel layout fixed after first call"

    w1a_p = np.zeros((D_PAD, H1), np.float32)
    w1a_p[:D] = W1a
    w1b_p = np.zeros((D_PAD, H1), np.float32)
    w1b_p[:D] = W1b
    w1a_p = _round_fp32r(w1a_p)
    w1b_p = _round_fp32r(w1b_p)
    b1c = b1.reshape(H1, 1)

    com = {
        "w1a": w1a_p,
        "w1b": w1b_p,
        "b1c": b1c,
    }
    w3f = W3.reshape(H2)
    if path == "v4":
        # fold |w3| into W2 columns (relu(c*x) = c*relu(x), c >= 0), sort
        # columns so w3 >= 0 comes first; sign row handles the subtraction
        perm = np.argsort(w3f < 0, kind="stable")
        pos = int((w3f >= 0).sum())
        w2s = (W2 * np.abs(w3f)[None, :])[:, perm]
        sgn = np.sign(w3f)[perm]
        com["w2s"] = _to_bf16(w2s)
        com["sgn"] = np.ascontiguousarray(
            np.broadcast_to(sgn, (128, H2)), dtype=np.float32)
        _POS_CNT = pos
    else:
        com["w2"] = _to_bf16(W2)
        com["w3"] = _to_bf16(W3.reshape(H2, 1))
        com["b2c"] = b2.reshape(H2, 1)

    per_core = []
    for c in range(N_CORES):
        b = c // 4
        n0 = (c % 4) * N_LOC
        pa = np.zeros((D_PAD, N_LOC), np.float32)
        pa[:D] = Phi_A[b, n0:n0 + N_LOC, :].T
        pb = np.zeros((D_PAD, M), np.float32)
        pb[:D] = Phi_B[b].T
        per_core.append(dict(com, pa_t=_round_fp32r(pa), pb_t=_round_fp32r(pb)))
    return per_core


def kernel(Phi_A, Phi_B, W1a, W1b, b1, W2, b2, W3, b3):
    b3 = np.asarray(b3, dtype=np.float32)
    per_core = build_per_core({
        "Phi_A": Phi_A, "Phi_B": Phi_B, "W1a": W1a, "W1b": W1b,
        "b1": b1, "W2": W2, "b2": b2, "W3": W3,
    })
    runner = _get_runner()
    outs = runner.run(per_core)
    dro_flat = outs[runner.out_names.index("dro")]
    dro_flat = np.asarray(dro_flat).reshape(N_CORES, N_LOC * M)
    dro = np.empty((B, N, M), np.float32)
    for c in range(N_CORES):
        b = c // 4
        n0 = (c % 4) * N_LOC
        dro[b, n0:n0 + N_LOC, :] = dro_flat[c].reshape(N_LOC, M)
    return dro + b3.reshape(-1)[0]
